# revision 12
# baseline (speedup 1.0000x reference)
"""Trainium2 Bass kernel for CustomHyperSemanticMessagePassing (hypergraph
multi-head single-query attention message passing).

Math (reference):
  Wh = x @ w_lin ; We = edge_attr @ w_e
  q  = (Wh @ w_q + b_q)/sqrt(dh)               per node, [N,H,dh]
  k_p = (Wh[u] + We[e]) @ w_k + b_k            per pair (v,e,u)
  v_p = Wh[u] @ w_v + b_v
  scores_p = <q[v], k_p> per head ; segmented softmax over each node v's pairs
  out = relu(segsum(alpha * v_p) @ w_o + b_o)

Kernel strategy (8 NeuronCores, SPMD, no collectives):
  * Algebraic refactor: fold w_lin into the q/k/v projections so no per-pair
    matmuls are needed:
      KV table  [N,256] = [x @ (w_lin w_k) | x @ (w_lin w_v) + b_v]
      KE table  [E,128] =  edge_attr @ (w_e w_k) + b_k
      q table   [slots,128] = (x_perm @ (w_lin w_q) + b_q)/4, block-ordered
    k_p = KV[u,:128] + KE[e]; v_p = KV[u,128:].
  * Every core builds the full KV/KE tables (the tables must land in every
    core's HBM anyway; recompute is cheaper than collectives) and the q table
    for its own 1/8 node slab, with q rows pre-permuted to block order.
  * Pairs are routed (host side) to the core owning v, sorted by owner, and
    bin-packed into blocks of <=128 owner nodes / <=1024 pairs. Per 128-pair
    tile the device gathers KV/KE rows with indirect DMA, expands q via a
    one-hot matmul (one-hot built with iota+is_equal, transposed on PE),
    computes scores with vector ops, exp on ScalarE (scores are O(1),
    max-subtraction provably unnecessary), and segment-reduces
    numerator+denominator with a single one-hot scatter matmul accumulated in
    PSUM across the block's 8 tiles.
  * One-hot matrices are exact in bf16, so the expansion/scatter matmuls run
    in bf16 (single-pass on the fp32-double-pumped PE); accumulation stays
    fp32 in PSUM.
  * Host un-permutes the per-core block-major outputs into the final [N,128].
"""

import math
import numpy as np
from contextlib import ExitStack

import concourse.bass as bass
import concourse.bacc as bacc
import concourse.tile as tile
import concourse.mybir as mybir
from concourse.bass_utils import run_bass_kernel_spmd
from concourse.masks import make_identity

F32 = mybir.dt.float32
BF16 = mybir.dt.bfloat16
I32 = mybir.dt.int32

N, E, D, ED, H = 100000, 50000, 128, 64, 8
DH = D // H
NC = 8
NLOC = N // NC
P128 = 128
TPB = 8                  # 128-pair tiles per block
CAP = TPB * P128         # pairs per block
CAP_NODES = 128          # owner nodes per block (M-matrix columns)
PAD_BIAS = -30000.0      # additive score bias for padding pairs -> exp == 0
ABATCH = 8               # phase-A tiles per DMA batch

NTX = -(-N // P128)                  # 782 x tiles
NTX4 = -(-NTX // ABATCH) * ABATCH    # padded to batch -> 784
PADN = NTX4 * P128
NTE = -(-E // P128)                  # 391 edge tiles
NTE4 = -(-NTE // ABATCH) * ABATCH    # 392
PADE = NTE4 * P128


# ----------------------------------------------------------------------------
# host-side routing
# ----------------------------------------------------------------------------

def _pack_nodes(cnt, nblk):
    """Worst-fit-decreasing packing of nodes into nblk blocks with
    <=CAP_NODES nodes and <=CAP pairs each. Returns list of node-id lists or
    None if it does not fit."""
    import heapq
    order = np.argsort(-cnt, kind="stable")
    heap = [(-CAP, bi) for bi in range(nblk)]
    heapq.heapify(heap)
    nodes = [[] for _ in range(nblk)]
    for nid in order:
        c = int(cnt[nid])
        placed = False
        while heap:
            negrem, bi = heapq.heappop(heap)
            rem = -negrem
            if rem < c:
                heapq.heappush(heap, (negrem, bi))
                break
            nodes[bi].append(nid)
            if len(nodes[bi]) < CAP_NODES:
                heapq.heappush(heap, (-(rem - c), bi))
            placed = True
            break
        if not placed:
            return None
    return nodes


def _route(owners, pair_e, pair_u):
    """Sort pairs by owner, split per core, pack blocks, build device arrays.

    Returns (nblk, per_core_arrays, node_map). per_core_arrays[c] =
    (ints [nblk,128,16]  : pair_u cols 0..7, pair_e cols 8..15, by tile,
     floats [nblk,128,16]: owner col-index cols 0..7, pad bias cols 8..15,
     qcols [nblk*128]    : global x row for each block column, -1 for pad).
    node_map[c] maps output row -> global node id (-1 for padding)."""
    perm = np.argsort(owners, kind="stable")
    o_s = owners[perm]
    e_s = pair_e[perm]
    u_s = pair_u[perm]
    bounds = np.searchsorted(o_s, np.arange(NC + 1) * NLOC)

    packs = []
    nblk = 0
    for c in range(NC):
        lo, hi = int(bounds[c]), int(bounds[c + 1])
        loc = o_s[lo:hi] - c * NLOC
        cnt = np.bincount(loc, minlength=NLOC)
        starts = np.zeros(NLOC + 1, np.int64)
        np.cumsum(cnt, out=starts[1:])
        nb = max(math.ceil(NLOC / CAP_NODES), math.ceil((hi - lo) / CAP))
        while True:
            nodes = _pack_nodes(cnt, nb)
            if nodes is not None:
                break
            nb += 1
        packs.append((lo, nodes, cnt, starts))
        nblk = max(nblk, nb)

    import ml_dtypes
    bf = ml_dtypes.bfloat16
    NSLOT = CAP // 4
    per_core = []
    maps = []
    for c in range(NC):
        lo, nodes, cnt, starts = packs[c]
        ints = np.zeros((nblk, P128, 10), np.int32)   # 0..7 pair_u, 8..9 slot_e
        # mm8: [m8 (8x128) | mt8 (8x128) | bias (8)] along the free dim
        mm8 = np.zeros((nblk, P128, 2 * TPB * P128 + 8), bf)
        m8 = mm8[:, :, 0:TPB * P128].reshape(nblk, P128, TPB, P128)
        mt8 = mm8[:, :, TPB * P128:2 * TPB * P128].reshape(nblk, P128, TPB, P128)
        bias8 = mm8[:, :, 2 * TPB * P128:]
        qcols = np.full(nblk * P128, -1, np.int64)
        nmap = np.full(nblk * P128, -1, np.int64)
        for b, blk in enumerate(nodes):
            fu = np.zeros(CAP, np.int32)
            fe = np.zeros(CAP, np.int32)
            frel = np.zeros(CAP, np.int64)
            fbias = np.full(CAP, PAD_BIAS, np.float32)
            pos = 0
            for j, nid in enumerate(blk):
                s0 = lo + int(starts[nid])
                k = int(cnt[nid])
                fu[pos:pos + k] = u_s[s0:s0 + k]
                fe[pos:pos + k] = e_s[s0:s0 + k]
                frel[pos:pos + k] = j
                fbias[pos:pos + k] = 0.0
                qcols[b * P128 + j] = c * NLOC + nid
                nmap[b * P128 + j] = c * NLOC + nid
                pos += k
            ints[b, :, 0:8] = fu.reshape(TPB, P128).T
            ints[b, :, 8:10] = fe[::4].reshape(2, NSLOT // 2).T.reshape(P128, 2)
            bias8[b] = fbias.reshape(TPB, P128).T
            onehot = (frel[:, None] == np.arange(P128)[None, :]).astype(bf)
            oh3 = onehot.reshape(TPB, P128, P128)
            m8[b] = oh3.transpose(1, 0, 2)      # [pair_p, t, col_j]
            mt8[b] = oh3.transpose(2, 0, 1)     # [col_c, t, pair_p]
        per_core.append((ints, mm8, qcols))
        maps.append(nmap)
    return nblk, per_core, np.stack(maps)


# ----------------------------------------------------------------------------
# device program
# ----------------------------------------------------------------------------

def _build_nc(nblk):
    nc = bacc.Bacc()
    ntq = -(-nblk // ABATCH) * ABATCH      # q tiles (one per block), padded
    padq = ntq * P128

    xT = nc.declare_dram_parameter("xT", [P128, PADN], BF16, isOutput=False)
    xqT = nc.declare_dram_parameter("xqT", [P128, padq], BF16, isOutput=False)
    eaT = nc.declare_dram_parameter("eaT", [ED, PADE], BF16, isOutput=False)
    w_linT = nc.declare_dram_parameter("w_linT", [D, D], F32, isOutput=False)
    w_eT = nc.declare_dram_parameter("w_eT", [D, ED], F32, isOutput=False)
    w_q = nc.declare_dram_parameter("w_q", [D, D], F32, isOutput=False)
    w_k = nc.declare_dram_parameter("w_k", [D, D], F32, isOutput=False)
    w_v = nc.declare_dram_parameter("w_v", [D, D], F32, isOutput=False)
    w_o = nc.declare_dram_parameter("w_o", [D, D], F32, isOutput=False)
    bv_c = nc.declare_dram_parameter("bv_c", [D, 1], F32, isOutput=False)
    bq_m = nc.declare_dram_parameter("bq_m", [P128, D], F32, isOutput=False)
    bo_m = nc.declare_dram_parameter("bo_m", [P128, D], F32, isOutput=False)
    ints_p = nc.declare_dram_parameter("ints_p", [nblk, P128, 10], I32, isOutput=False)
    mm8_p = nc.declare_dram_parameter("mm8_p", [nblk, P128, 2 * TPB * P128 + 8], BF16, isOutput=False)
    r4_p = nc.declare_dram_parameter("r4_p", [P128, 4 * P128], BF16, isOutput=False)
    out = nc.declare_dram_parameter("out", [nblk * P128, D], F32, isOutput=True)

    kv_d = nc.dram_tensor("kv_d", [PADN, 2 * D], BF16)
    ke_d = nc.dram_tensor("ke_d", [PADE, D], BF16)
    q_d = nc.dram_tensor("q_d", [padq, D], BF16)

    with ExitStack() as ctx:
        tc = ctx.enter_context(tile.TileContext(nc))
        consts = ctx.enter_context(tc.tile_pool(name="consts", bufs=1))

        ident_bf = consts.tile([P128, P128], BF16)
        make_identity(nc, ident_bf[:])
        ident_f = consts.tile([P128, P128], F32)
        make_identity(nc, ident_f[:])
        ones_row_f = consts.tile([1, P128], F32)
        nc.vector.memset(ones_row_f[:], 1.0)
        r4_sb = consts.tile([P128, 4 * P128], BF16)
        nc.sync.dma_start(out=r4_sb[:], in_=r4_p[:, :])

        # ---- load weights / fold projections --------------------------------
        wlt_sb = consts.tile([D, D], F32)
        nc.sync.dma_start(out=wlt_sb[:], in_=w_linT[:, :])
        wet_sb = consts.tile([D, ED], F32)
        nc.sync.dma_start(out=wet_sb[:], in_=w_eT[:, :])
        wq_sb = consts.tile([D, D], F32)
        nc.sync.dma_start(out=wq_sb[:], in_=w_q[:, :])
        wk_sb = consts.tile([D, D], F32)
        nc.sync.dma_start(out=wk_sb[:], in_=w_k[:, :])
        wv_sb = consts.tile([D, D], F32)
        nc.sync.dma_start(out=wv_sb[:], in_=w_v[:, :])
        wo_sb = consts.tile([D, D], F32)
        nc.sync.dma_start(out=wo_sb[:], in_=w_o[:, :])
        bv_sb1 = consts.tile([D, 1], F32)
        nc.sync.dma_start(out=bv_sb1[:], in_=bv_c[:, :])
        bq_raw = consts.tile([P128, D], F32)
        nc.sync.dma_start(out=bq_raw[:], in_=bq_m[:, :])
        bo_sb = consts.tile([P128, D], F32)
        nc.sync.dma_start(out=bo_sb[:], in_=bo_m[:, :])
        bq_sb = consts.tile([P128, D], F32)
        nc.scalar.mul(out=bq_sb[:], in_=bq_raw[:], mul=1.0 / math.sqrt(DH))

        w_cat = consts.tile([D, 2 * D], BF16)     # [w_lin w_k | w_lin w_v]
        w_lq = consts.tile([D, D], BF16)          # (w_lin w_q)/sqrt(dh)
        w_ek = consts.tile([ED, D], BF16)         # w_e w_k

        with tc.tile_pool(name="ps0", bufs=1, space="PSUM") as ps0:
            wcat_ps = ps0.tile([D, 2 * D], F32, space="PSUM")
            nc.tensor.matmul(out=wcat_ps[:, 0:D], lhsT=wlt_sb[:], rhs=wk_sb[:],
                             start=True, stop=True)
            nc.tensor.matmul(out=wcat_ps[:, D:2 * D], lhsT=wlt_sb[:], rhs=wv_sb[:],
                             start=True, stop=True)
            nc.vector.tensor_copy(w_cat[:], wcat_ps[:])
            wlq_ps = ps0.tile([D, D], F32, space="PSUM")
            nc.tensor.matmul(out=wlq_ps[:], lhsT=wlt_sb[:], rhs=wq_sb[:],
                             start=True, stop=True)
            nc.scalar.mul(out=w_lq[:], in_=wlq_ps[:], mul=1.0 / math.sqrt(DH))
            wek_ps = ps0.tile([ED, D], F32, space="PSUM")
            nc.tensor.matmul(out=wek_ps[:], lhsT=wet_sb[:], rhs=wk_sb[:],
                             start=True, stop=True)
            nc.vector.tensor_copy(w_ek[:], wek_ps[:])
            # bo2 = b_o + b_v @ w_o  (b_v shifts ctx by a constant since the
            # softmax weights sum to 1; push it through w_o once)
            bvo_ps = ps0.tile([1, D], F32, space="PSUM")
            nc.tensor.matmul(out=bvo_ps[:], lhsT=bv_sb1[:], rhs=wo_sb[:],
                             start=True, stop=True)
            bvo_sb = consts.tile([1, D], F32)
            nc.vector.tensor_copy(bvo_sb[:], bvo_ps[:])
            bvrep_ps = ps0.tile([P128, D], F32, space="PSUM")
            nc.tensor.matmul(out=bvrep_ps[:], lhsT=ones_row_f[:], rhs=bvo_sb[:],
                             start=True, stop=True)
            bo2_sb = consts.tile([P128, D], F32)
            nc.vector.tensor_add(bo2_sb[:], bvrep_ps[:], bo_sb[:])

        # ---- phase A: build KV / KE / q tables (4x-batched DMA) -------------
        with tc.tile_pool(name="sbA", bufs=4) as sbA, \
             tc.tile_pool(name="psA", bufs=4, space="PSUM") as psA:
            for i in range(NTX4 // ABATCH):
                xt4 = sbA.tile([P128, ABATCH * P128], BF16, tag="xt4")
                nc.sync.dma_start(
                    out=xt4[:], in_=xT[:, i * ABATCH * P128:(i + 1) * ABATCH * P128])
                kv4 = sbA.tile([P128, ABATCH, 2 * D], BF16, tag="kv4")
                for k in range(ABATCH):
                    mm = psA.tile([P128, 2 * D], F32, space="PSUM", tag="psa")
                    nc.tensor.matmul(out=mm[:], lhsT=xt4[:, k * P128:(k + 1) * P128],
                                     rhs=w_cat[:], start=True, stop=True)
                    if k % 2 == 0:
                        nc.scalar.copy(out=kv4[:, k, :], in_=mm[:])
                    else:
                        nc.vector.tensor_copy(kv4[:, k, :], mm[:])
                dst = kv_d[i * ABATCH * P128:(i + 1) * ABATCH * P128, :]
                nc.sync.dma_start(
                    out=dst.rearrange("(k p) w -> p k w", p=P128), in_=kv4[:])

            for i in range(NTE4 // ABATCH):
                et4 = sbA.tile([ED, ABATCH * P128], BF16, tag="et4")
                nc.sync.dma_start(
                    out=et4[:], in_=eaT[:, i * ABATCH * P128:(i + 1) * ABATCH * P128])
                ke4 = sbA.tile([P128, ABATCH, D], BF16, tag="ke4")
                for k in range(ABATCH):
                    kem_full = psA.tile([P128, 2 * D], F32, space="PSUM", tag="psa")
                    kem = kem_full[:, 0:D]
                    nc.tensor.matmul(out=kem, lhsT=et4[:, k * P128:(k + 1) * P128],
                                     rhs=w_ek[:], start=True, stop=True)
                    if k % 2 == 0:
                        nc.scalar.copy(out=ke4[:, k, :], in_=kem)
                    else:
                        nc.vector.tensor_copy(ke4[:, k, :], kem)
                dst = ke_d[i * ABATCH * P128:(i + 1) * ABATCH * P128, :]
                nc.sync.dma_start(
                    out=dst.rearrange("(k p) w -> p k w", p=P128), in_=ke4[:])

            for i in range(ntq // ABATCH):
                xq4 = sbA.tile([P128, ABATCH * P128], BF16, tag="xq4")
                nc.sync.dma_start(
                    out=xq4[:], in_=xqT[:, i * ABATCH * P128:(i + 1) * ABATCH * P128])
                q4 = sbA.tile([P128, ABATCH, D], BF16, tag="q4")
                for k in range(ABATCH):
                    qm_full = psA.tile([P128, 2 * D], F32, space="PSUM", tag="psa")
                    qm = qm_full[:, 0:D]
                    nc.tensor.matmul(out=qm, lhsT=xq4[:, k * P128:(k + 1) * P128],
                                     rhs=w_lq[:], start=True, stop=True)
                    nc.vector.tensor_add(q4[:, k, :], qm, bq_sb[:])
                dst = q_d[i * ABATCH * P128:(i + 1) * ABATCH * P128, :]
                nc.sync.dma_start(
                    out=dst.rearrange("(k p) w -> p k w", p=P128), in_=q4[:])

        tc.strict_bb_all_engine_barrier()

        # ---- phase B: per-block gather + attention + segment reduce ---------
        MOFF = 0                    # m8 offset in mm8
        TOFF = TPB * P128           # mt8 offset
        BOFF = 2 * TPB * P128       # bias offset
        G4 = 4                      # tiles per DVE batch group
        with tc.tile_pool(name="sbB", bufs=4) as sbB, \
             tc.tile_pool(name="sbB2", bufs=3) as sbB2, \
             tc.tile_pool(name="ps_ke", bufs=2, space="PSUM") as ps_ke, \
             tc.tile_pool(name="ps_q", bufs=2, space="PSUM") as ps_q, \
             tc.tile_pool(name="ps_acc", bufs=2, space="PSUM") as ps_acc, \
             tc.tile_pool(name="ps_epi", bufs=2, space="PSUM") as ps_epi:
            for b in range(nblk):
                ints_sb = sbB2.tile([P128, 10], I32, tag="ints")
                nc.sync.dma_start(out=ints_sb[:], in_=ints_p[b, :, :])
                mm8_sb = sbB2.tile([P128, 2 * TPB * P128 + 8], BF16, tag="mm8")
                nc.sync.dma_start(out=mm8_sb[:], in_=mm8_p[b, :, :])
                qblk = sbB2.tile([P128, D], BF16, tag="qblk")
                nc.sync.dma_start(out=qblk[:], in_=q_d[b * P128:(b + 1) * P128, :])
                keS0 = sbB2.tile([P128, D], BF16, tag="keS0")
                nc.gpsimd.indirect_dma_start(
                    out=keS0[:], out_offset=None, in_=ke_d[:, :],
                    in_offset=bass.IndirectOffsetOnAxis(ap=ints_sb[:, 8:9], axis=0))
                keS1 = sbB2.tile([P128, D], BF16, tag="keS1")
                nc.gpsimd.indirect_dma_start(
                    out=keS1[:], out_offset=None, in_=ke_d[:, :],
                    in_offset=bass.IndirectOffsetOnAxis(ap=ints_sb[:, 9:10], axis=0))

                acc = ps_acc.tile([P128, D + H], F32, space="PSUM", tag="acc")
                for g in range(TPB // G4):
                    kv_g4 = sbB.tile([P128, G4, 2 * D], BF16, tag="kv_g4")
                    for i in range(G4):
                        t = g * G4 + i
                        nc.gpsimd.indirect_dma_start(
                            out=kv_g4[:, i, :], out_offset=None, in_=kv_d[:, :],
                            in_offset=bass.IndirectOffsetOnAxis(ap=ints_sb[:, t:t + 1], axis=0))
                    keX4 = ps_ke.tile([P128, G4, D], F32, space="PSUM", tag="keX4")
                    qx4 = ps_q.tile([P128, G4, D], F32, space="PSUM", tag="qx4")
                    keS = keS0 if g == 0 else keS1
                    for i in range(G4):
                        t = g * G4 + i
                        nc.tensor.matmul(
                            out=keX4[:, i, :],
                            lhsT=r4_sb[:, (t % 4) * P128:(t % 4 + 1) * P128],
                            rhs=keS[:], start=True, stop=True)
                        nc.tensor.matmul(
                            out=qx4[:, i, :],
                            lhsT=mm8_sb[:, TOFF + t * P128:TOFF + (t + 1) * P128],
                            rhs=qblk[:], start=True, stop=True)

                    kk4 = sbB.tile([P128, G4, D], F32, tag="kk4")
                    nc.vector.tensor_add(kk4[:], kv_g4[:, :, 0:D], keX4[:])
                    prod4 = sbB.tile([P128, G4, D], F32, tag="prod4")
                    nc.vector.tensor_mul(prod4[:], kk4[:], qx4[:])
                    sc4 = sbB.tile([P128, G4 * H], F32, tag="sc4")
                    nc.vector.tensor_reduce(
                        out=sc4[:], in_=prod4[:].rearrange("p g (h d) -> p (g h) d", h=H),
                        axis=mybir.AxisListType.X, op=mybir.AluOpType.add)
                    scb4 = sbB.tile([P128, G4, H], F32, tag="scb4")
                    nc.vector.tensor_tensor(
                        out=scb4[:], in0=sc4[:].rearrange("p (g h) -> p g h", g=G4),
                        in1=mm8_sb[:, BOFF + g * G4:BOFF + (g + 1) * G4]
                            .rearrange("p (g o) -> p g o", o=1).to_broadcast([P128, G4, H]),
                        op=mybir.AluOpType.add)
                    wex4 = sbB.tile([P128, G4, D + H], BF16, tag="wex4")
                    ex4 = wex4[:, :, D:D + H]
                    nc.scalar.activation(
                        out=ex4, in_=scb4[:], func=mybir.ActivationFunctionType.Exp,
                        bias=0.0, scale=1.0)
                    nc.vector.tensor_tensor(
                        out=wex4[:, :, 0:D].rearrange("p g (h d) -> p g h d", h=H),
                        in0=kv_g4[:, :, D:2 * D].rearrange("p g (h d) -> p g h d", h=H),
                        in1=ex4.rearrange("p g (h o) -> p g h o", o=1)
                            .to_broadcast([P128, G4, H, DH]),
                        op=mybir.AluOpType.mult)

                    for i in range(G4):
                        t = g * G4 + i
                        # single matmul per tile: PSUM `start` clears
                        # has_written bank-wide -> NUM+DEN in one group
                        nc.tensor.matmul(
                            out=acc[:],
                            lhsT=mm8_sb[:, MOFF + t * P128:MOFF + (t + 1) * P128],
                            rhs=wex4[:, i, :], start=(t == 0), stop=(t == TPB - 1))

                den = sbB2.tile([P128, H], F32, tag="den")
                nc.vector.tensor_scalar_add(out=den[:], in0=acc[:, D:D + H],
                                            scalar1=1e-30)
                denr = sbB2.tile([P128, H], F32, tag="denr")
                nc.vector.reciprocal(denr[:], den[:])
                ctx_sb = sbB2.tile([P128, D], F32, tag="ctx")
                nc.vector.tensor_tensor(
                    out=ctx_sb[:].rearrange("p (h d) -> p h d", h=H),
                    in0=acc[:, 0:D].rearrange("p (h d) -> p h d", h=H),
                    in1=denr[:].rearrange("p (h o) -> p h o", o=1).to_broadcast([P128, H, DH]),
                    op=mybir.AluOpType.mult)
                ctxT_ps = ps_epi.tile([P128, D], F32, space="PSUM", tag="epi")
                nc.tensor.transpose(out=ctxT_ps[:], in_=ctx_sb[:], identity=ident_f[:])
                ctxT_sb = sbB2.tile([P128, D], F32, tag="ctxT_sb")
                nc.scalar.copy(out=ctxT_sb[:], in_=ctxT_ps[:])
                o_ps = ps_epi.tile([P128, D], F32, space="PSUM", tag="epi")
                nc.tensor.matmul(out=o_ps[:], lhsT=ctxT_sb[:], rhs=wo_sb[:],
                                 start=True, stop=True)
                o_sb = sbB2.tile([P128, D], F32, tag="o_sb")
                nc.vector.tensor_add(o_sb[:], o_ps[:], bo2_sb[:])
                o_relu = sbB2.tile([P128, D], F32, tag="o_relu")
                nc.scalar.activation(out=o_relu[:], in_=o_sb[:],
                                     func=mybir.ActivationFunctionType.Relu)
                nc.sync.dma_start(out=out[b * P128:(b + 1) * P128, :], in_=o_relu[:])

    nc.compile()
    return nc


_CACHE = {}


def _get_nc(nblk):
    if nblk not in _CACHE:
        _CACHE[nblk] = _build_nc(nblk)
    return _CACHE[nblk]


def kernel(**inputs):
    import ml_dtypes
    x = np.ascontiguousarray(np.asarray(inputs["x"], np.float32))
    ea = np.ascontiguousarray(np.asarray(inputs["edge_attr"], np.float32))
    owners = np.asarray(inputs["owners"], np.int32)
    pair_e = np.asarray(inputs["pair_e"], np.int32)
    pair_u = np.asarray(inputs["pair_u"], np.int32)

    nblk, per_core, node_map = _route(owners, pair_e, pair_u)
    nc = _get_nc(nblk)
    ntq = -(-nblk // ABATCH) * ABATCH
    padq = ntq * P128

    bf = ml_dtypes.bfloat16
    xT = np.zeros((P128, PADN), bf)
    xT[:, :N] = x.T.astype(bf)
    eaT = np.zeros((ED, PADE), bf)
    eaT[:, :E] = ea.T.astype(bf)

    def trep(b):
        return np.tile(np.asarray(b, np.float32)[None, :], (P128, 1))

    shared = dict(
        xT=xT, eaT=eaT,
        w_linT=np.ascontiguousarray(np.asarray(inputs["w_lin"], np.float32).T),
        w_eT=np.ascontiguousarray(np.asarray(inputs["w_e"], np.float32).T),
        w_q=np.asarray(inputs["w_q"], np.float32),
        w_k=np.asarray(inputs["w_k"], np.float32),
        w_v=np.asarray(inputs["w_v"], np.float32),
        w_o=np.asarray(inputs["w_o"], np.float32),
        bv_c=np.asarray(inputs["b_v"], np.float32).reshape(D, 1),
        bq_m=trep(inputs["b_q"]), bo_m=trep(inputs["b_o"]),
    )
    # R4[s, t*128 + j] = 1 iff s == 32*t + j//4  (slot -> pair expansion)
    r4 = np.zeros((P128, 4, P128), np.float32)
    jj = np.arange(P128)
    for tm in range(4):
        r4[32 * tm + jj // 4, tm, jj] = 1.0
    r4 = r4.reshape(P128, 4 * P128).astype(bf)
    in_maps = []
    for c in range(NC):
        ints, mm8, qcols = per_core[c]
        xqT = np.zeros((P128, padq), bf)
        valid = qcols >= 0
        xqT[:, np.nonzero(valid)[0]] = x[qcols[valid]].T.astype(bf)
        in_maps.append(dict(shared, xqT=xqT, ints_p=ints, mm8_p=mm8, r4_p=r4))

    import os
    trace = os.environ.get("KERNEL_TRACE", "0") == "1"
    kwargs = {}
    if trace:
        kwargs = dict(trace=True, tmpdir=os.environ.get("KERNEL_TRACE_DIR") or None)
    res = run_bass_kernel_spmd(nc, in_maps, core_ids=list(range(NC)), **kwargs)
    global _LAST_RESULTS
    _LAST_RESULTS = res

    out_full = np.zeros((N, D), np.float32)
    for c in range(NC):
        oc = res.results[c]["out"]
        valid = node_map[c] >= 0
        out_full[node_map[c][valid]] = oc[valid]
    return out_full


# revision 13
# speedup vs baseline: 1.1359x; 1.1359x over previous
"""Trainium2 Bass kernel for CustomHyperSemanticMessagePassing (hypergraph
multi-head single-query attention message passing).

Math (reference):
  Wh = x @ w_lin ; We = edge_attr @ w_e
  q  = (Wh @ w_q + b_q)/sqrt(dh)               per node, [N,H,dh]
  k_p = (Wh[u] + We[e]) @ w_k + b_k            per pair (v,e,u)
  v_p = Wh[u] @ w_v + b_v
  scores_p = <q[v], k_p> per head ; segmented softmax over each node v's pairs
  out = relu(segsum(alpha * v_p) @ w_o + b_o)

Kernel strategy (8 NeuronCores, SPMD, no collectives):
  * Algebraic refactor: fold w_lin into the q/k/v projections so no per-pair
    matmuls are needed:
      KV table  [N,256] = [x @ (w_lin w_k) | x @ (w_lin w_v) + b_v]
      KE table  [E,128] =  edge_attr @ (w_e w_k) + b_k
      q table   [slots,128] = (x_perm @ (w_lin w_q) + b_q)/4, block-ordered
    k_p = KV[u,:128] + KE[e]; v_p = KV[u,128:].
  * Every core builds the full KV/KE tables (the tables must land in every
    core's HBM anyway; recompute is cheaper than collectives) and the q table
    for its own 1/8 node slab, with q rows pre-permuted to block order.
  * Pairs are routed (host side) to the core owning v, sorted by owner, and
    bin-packed into blocks of <=128 owner nodes / <=1024 pairs. Per 128-pair
    tile the device gathers KV/KE rows with indirect DMA, expands q via a
    one-hot matmul (one-hot built with iota+is_equal, transposed on PE),
    computes scores with vector ops, exp on ScalarE (scores are O(1),
    max-subtraction provably unnecessary), and segment-reduces
    numerator+denominator with a single one-hot scatter matmul accumulated in
    PSUM across the block's 8 tiles.
  * One-hot matrices are exact in bf16, so the expansion/scatter matmuls run
    in bf16 (single-pass on the fp32-double-pumped PE); accumulation stays
    fp32 in PSUM.
  * Host un-permutes the per-core block-major outputs into the final [N,128].
"""

import math
import numpy as np
from contextlib import ExitStack

import concourse.bass as bass
import concourse.bacc as bacc
import concourse.tile as tile
import concourse.mybir as mybir
from concourse.bass_utils import run_bass_kernel_spmd
from concourse.masks import make_identity

F32 = mybir.dt.float32
BF16 = mybir.dt.bfloat16
I32 = mybir.dt.int32

N, E, D, ED, H = 100000, 50000, 128, 64, 8
DH = D // H
NC = 8
NLOC = N // NC
P128 = 128
TPB = 8                  # 128-pair tiles per block
CAP = TPB * P128         # pairs per block
CAP_NODES = 128          # owner nodes per block (M-matrix columns)
PAD_BIAS = -30000.0      # additive score bias for padding pairs -> exp == 0
ABATCH = 8               # phase-A tiles per DMA batch

NTX = -(-N // P128)                  # 782 x tiles
NTX4 = -(-NTX // ABATCH) * ABATCH    # padded to batch -> 784
PADN = NTX4 * P128
NTE = -(-E // P128)                  # 391 edge tiles
NTE4 = -(-NTE // ABATCH) * ABATCH    # 392
PADE = NTE4 * P128


# ----------------------------------------------------------------------------
# host-side routing
# ----------------------------------------------------------------------------

def _pack_nodes(cnt, nblk):
    """Worst-fit-decreasing packing of nodes into nblk blocks with
    <=CAP_NODES nodes and <=CAP pairs each. Returns list of node-id lists or
    None if it does not fit."""
    import heapq
    order = np.argsort(-cnt, kind="stable")
    heap = [(-CAP, bi) for bi in range(nblk)]
    heapq.heapify(heap)
    nodes = [[] for _ in range(nblk)]
    for nid in order:
        c = int(cnt[nid])
        placed = False
        while heap:
            negrem, bi = heapq.heappop(heap)
            rem = -negrem
            if rem < c:
                heapq.heappush(heap, (negrem, bi))
                break
            nodes[bi].append(nid)
            if len(nodes[bi]) < CAP_NODES:
                heapq.heappush(heap, (-(rem - c), bi))
            placed = True
            break
        if not placed:
            return None
    return nodes


def _route(owners, pair_e, pair_u):
    """Sort pairs by owner, split per core, pack blocks, build device arrays.

    Returns (nblk, per_core_arrays, node_map). per_core_arrays[c] =
    (ints [nblk,128,16]  : pair_u cols 0..7, pair_e cols 8..15, by tile,
     floats [nblk,128,16]: owner col-index cols 0..7, pad bias cols 8..15,
     qcols [nblk*128]    : global x row for each block column, -1 for pad).
    node_map[c] maps output row -> global node id (-1 for padding)."""
    perm = np.argsort(owners, kind="stable")
    o_s = owners[perm]
    e_s = pair_e[perm]
    u_s = pair_u[perm]
    bounds = np.searchsorted(o_s, np.arange(NC + 1) * NLOC)

    packs = []
    nblk = 0
    for c in range(NC):
        lo, hi = int(bounds[c]), int(bounds[c + 1])
        loc = o_s[lo:hi] - c * NLOC
        cnt = np.bincount(loc, minlength=NLOC)
        starts = np.zeros(NLOC + 1, np.int64)
        np.cumsum(cnt, out=starts[1:])
        nb = max(math.ceil(NLOC / CAP_NODES), math.ceil((hi - lo) / CAP))
        while True:
            nodes = _pack_nodes(cnt, nb)
            if nodes is not None:
                break
            nb += 1
        packs.append((lo, nodes, cnt, starts))
        nblk = max(nblk, nb)

    import ml_dtypes
    bf = ml_dtypes.bfloat16
    NSLOT = CAP // 4
    per_core = []
    maps = []
    for c in range(NC):
        lo, nodes, cnt, starts = packs[c]
        ints = np.zeros((nblk, P128, 10), np.int32)   # 0..7 pair_u, 8..9 slot_e
        # mm8: [m8 (8x128) | mt8 (8x128) | bias (8)] along the free dim
        mm8 = np.zeros((nblk, P128, 2 * TPB * P128 + 8), bf)
        m8 = mm8[:, :, 0:TPB * P128].reshape(nblk, P128, TPB, P128)
        mt8 = mm8[:, :, TPB * P128:2 * TPB * P128].reshape(nblk, P128, TPB, P128)
        bias8 = mm8[:, :, 2 * TPB * P128:]
        qcols = np.full(nblk * P128, -1, np.int64)
        nmap = np.full(nblk * P128, -1, np.int64)
        for b, blk in enumerate(nodes):
            fu = np.zeros(CAP, np.int32)
            fe = np.zeros(CAP, np.int32)
            frel = np.zeros(CAP, np.int64)
            fbias = np.full(CAP, PAD_BIAS, np.float32)
            pos = 0
            for j, nid in enumerate(blk):
                s0 = lo + int(starts[nid])
                k = int(cnt[nid])
                fu[pos:pos + k] = u_s[s0:s0 + k]
                fe[pos:pos + k] = e_s[s0:s0 + k]
                frel[pos:pos + k] = j
                fbias[pos:pos + k] = 0.0
                qcols[b * P128 + j] = c * NLOC + nid
                nmap[b * P128 + j] = c * NLOC + nid
                pos += k
            ints[b, :, 0:8] = fu.reshape(TPB, P128).T
            ints[b, :, 8:10] = fe[::4].reshape(2, NSLOT // 2).T.reshape(P128, 2)
            bias8[b] = fbias.reshape(TPB, P128).T
            onehot = (frel[:, None] == np.arange(P128)[None, :]).astype(bf)
            oh3 = onehot.reshape(TPB, P128, P128)
            m8[b] = oh3.transpose(1, 0, 2)      # [pair_p, t, col_j]
            mt8[b] = oh3.transpose(2, 0, 1)     # [col_c, t, pair_p]
        per_core.append((ints, mm8, qcols))
        maps.append(nmap)
    return nblk, per_core, np.stack(maps)


# ----------------------------------------------------------------------------
# device program
# ----------------------------------------------------------------------------

def _build_nc(nblk):
    nc = bacc.Bacc()
    ntq = -(-nblk // ABATCH) * ABATCH      # q tiles (one per block), padded
    padq = ntq * P128

    xT = nc.declare_dram_parameter("xT", [P128, PADN], BF16, isOutput=False)
    xqT = nc.declare_dram_parameter("xqT", [P128, padq], BF16, isOutput=False)
    eaT = nc.declare_dram_parameter("eaT", [ED, PADE], BF16, isOutput=False)
    w_linT = nc.declare_dram_parameter("w_linT", [D, D], F32, isOutput=False)
    w_eT = nc.declare_dram_parameter("w_eT", [D, ED], F32, isOutput=False)
    w_q = nc.declare_dram_parameter("w_q", [D, D], F32, isOutput=False)
    w_k = nc.declare_dram_parameter("w_k", [D, D], F32, isOutput=False)
    w_v = nc.declare_dram_parameter("w_v", [D, D], F32, isOutput=False)
    w_o = nc.declare_dram_parameter("w_o", [D, D], F32, isOutput=False)
    bv_c = nc.declare_dram_parameter("bv_c", [D, 1], F32, isOutput=False)
    bq_m = nc.declare_dram_parameter("bq_m", [P128, D], F32, isOutput=False)
    bo_m = nc.declare_dram_parameter("bo_m", [P128, D], F32, isOutput=False)
    ints_p = nc.declare_dram_parameter("ints_p", [nblk, P128, 10], I32, isOutput=False)
    mm8_p = nc.declare_dram_parameter("mm8_p", [nblk, P128, 2 * TPB * P128 + 8], BF16, isOutput=False)
    r4_p = nc.declare_dram_parameter("r4_p", [P128, 4 * P128], BF16, isOutput=False)
    out = nc.declare_dram_parameter("out", [nblk * P128, D], F32, isOutput=True)

    kv_d = nc.dram_tensor("kv_d", [PADN, 2 * D], BF16)
    ke_d = nc.dram_tensor("ke_d", [PADE, D], BF16)
    q_d = nc.dram_tensor("q_d", [padq, D], BF16)

    with ExitStack() as ctx:
        tc = ctx.enter_context(tile.TileContext(nc))
        consts = ctx.enter_context(tc.tile_pool(name="consts", bufs=1))

        ident_bf = consts.tile([P128, P128], BF16)
        make_identity(nc, ident_bf[:])
        ident_f = consts.tile([P128, P128], F32)
        make_identity(nc, ident_f[:])
        ones_row_f = consts.tile([1, P128], F32)
        nc.vector.memset(ones_row_f[:], 1.0)
        r4_sb = consts.tile([P128, 4 * P128], BF16)
        nc.sync.dma_start(out=r4_sb[:], in_=r4_p[:, :])
        keS_all = consts.tile([P128, 2 * nblk * P128], BF16)
        ints_all = consts.tile([P128, nblk, 10], I32)

        # ---- load weights / fold projections --------------------------------
        wlt_sb = consts.tile([D, D], F32)
        nc.sync.dma_start(out=wlt_sb[:], in_=w_linT[:, :])
        wet_sb = consts.tile([D, ED], F32)
        nc.sync.dma_start(out=wet_sb[:], in_=w_eT[:, :])
        wq_sb = consts.tile([D, D], F32)
        nc.sync.dma_start(out=wq_sb[:], in_=w_q[:, :])
        wk_sb = consts.tile([D, D], F32)
        nc.sync.dma_start(out=wk_sb[:], in_=w_k[:, :])
        wv_sb = consts.tile([D, D], F32)
        nc.sync.dma_start(out=wv_sb[:], in_=w_v[:, :])
        wo_sb = consts.tile([D, D], F32)
        nc.sync.dma_start(out=wo_sb[:], in_=w_o[:, :])
        bv_sb1 = consts.tile([D, 1], F32)
        nc.sync.dma_start(out=bv_sb1[:], in_=bv_c[:, :])
        bq_raw = consts.tile([P128, D], F32)
        nc.sync.dma_start(out=bq_raw[:], in_=bq_m[:, :])
        bo_sb = consts.tile([P128, D], F32)
        nc.sync.dma_start(out=bo_sb[:], in_=bo_m[:, :])
        bq_sb = consts.tile([P128, D], F32)
        nc.scalar.mul(out=bq_sb[:], in_=bq_raw[:], mul=1.0 / math.sqrt(DH))

        w_cat = consts.tile([D, 2 * D], BF16)     # [w_lin w_k | w_lin w_v]
        w_lq = consts.tile([D, D], BF16)          # (w_lin w_q)/sqrt(dh)
        w_ek = consts.tile([ED, D], BF16)         # w_e w_k

        with tc.tile_pool(name="ps0", bufs=1, space="PSUM") as ps0:
            wcat_ps = ps0.tile([D, 2 * D], F32, space="PSUM")
            nc.tensor.matmul(out=wcat_ps[:, 0:D], lhsT=wlt_sb[:], rhs=wk_sb[:],
                             start=True, stop=True)
            nc.tensor.matmul(out=wcat_ps[:, D:2 * D], lhsT=wlt_sb[:], rhs=wv_sb[:],
                             start=True, stop=True)
            nc.vector.tensor_copy(w_cat[:], wcat_ps[:])
            wlq_ps = ps0.tile([D, D], F32, space="PSUM")
            nc.tensor.matmul(out=wlq_ps[:], lhsT=wlt_sb[:], rhs=wq_sb[:],
                             start=True, stop=True)
            nc.scalar.mul(out=w_lq[:], in_=wlq_ps[:], mul=1.0 / math.sqrt(DH))
            wek_ps = ps0.tile([ED, D], F32, space="PSUM")
            nc.tensor.matmul(out=wek_ps[:], lhsT=wet_sb[:], rhs=wk_sb[:],
                             start=True, stop=True)
            nc.vector.tensor_copy(w_ek[:], wek_ps[:])
            # bo2 = b_o + b_v @ w_o  (b_v shifts ctx by a constant since the
            # softmax weights sum to 1; push it through w_o once)
            bvo_ps = ps0.tile([1, D], F32, space="PSUM")
            nc.tensor.matmul(out=bvo_ps[:], lhsT=bv_sb1[:], rhs=wo_sb[:],
                             start=True, stop=True)
            bvo_sb = consts.tile([1, D], F32)
            nc.vector.tensor_copy(bvo_sb[:], bvo_ps[:])
            bvrep_ps = ps0.tile([P128, D], F32, space="PSUM")
            nc.tensor.matmul(out=bvrep_ps[:], lhsT=ones_row_f[:], rhs=bvo_sb[:],
                             start=True, stop=True)
            bo2_sb = consts.tile([P128, D], F32)
            nc.vector.tensor_add(bo2_sb[:], bvrep_ps[:], bo_sb[:])

        # ---- phase A: build KV / KE / q tables (4x-batched DMA) -------------
        with tc.tile_pool(name="sbA", bufs=4) as sbA, \
             tc.tile_pool(name="psA", bufs=4, space="PSUM") as psA:
            for i in range(NTE4 // ABATCH):
                et4 = sbA.tile([ED, ABATCH * P128], BF16, tag="et4")
                nc.sync.dma_start(
                    out=et4[:], in_=eaT[:, i * ABATCH * P128:(i + 1) * ABATCH * P128])
                ke4 = sbA.tile([P128, ABATCH, D], BF16, tag="ke4")
                for k in range(ABATCH):
                    kem_full = psA.tile([P128, 2 * D], F32, space="PSUM", tag="psa")
                    kem = kem_full[:, 0:D]
                    nc.tensor.matmul(out=kem, lhsT=et4[:, k * P128:(k + 1) * P128],
                                     rhs=w_ek[:], start=True, stop=True)
                    if k % 2 == 0:
                        nc.scalar.copy(out=ke4[:, k, :], in_=kem)
                    else:
                        nc.vector.tensor_copy(ke4[:, k, :], kem)
                dst = ke_d[i * ABATCH * P128:(i + 1) * ABATCH * P128, :]
                nc.sync.dma_start(
                    out=dst.rearrange("(k p) w -> p k w", p=P128), in_=ke4[:])

            for i in range(ntq // ABATCH):
                xq4 = sbA.tile([P128, ABATCH * P128], BF16, tag="xq4")
                nc.sync.dma_start(
                    out=xq4[:], in_=xqT[:, i * ABATCH * P128:(i + 1) * ABATCH * P128])
                q4 = sbA.tile([P128, ABATCH, D], BF16, tag="q4")
                for k in range(ABATCH):
                    qm_full = psA.tile([P128, 2 * D], F32, space="PSUM", tag="psa")
                    qm = qm_full[:, 0:D]
                    nc.tensor.matmul(out=qm, lhsT=xq4[:, k * P128:(k + 1) * P128],
                                     rhs=w_lq[:], start=True, stop=True)
                    nc.vector.tensor_add(q4[:, k, :], qm, bq_sb[:])
                dst = q_d[i * ABATCH * P128:(i + 1) * ABATCH * P128, :]
                nc.sync.dma_start(
                    out=dst.rearrange("(k p) w -> p k w", p=P128), in_=q4[:])

            tc.strict_bb_all_engine_barrier()

            # ints for all blocks + early KE slot gathers: the slot gathers
            # only need ke_d/q_d, so GpSimd prefetches every block's KE rows
            # while the big KV table build (below) still runs on PE/ACT/DVE.
            nc.sync.dma_start(
                out=ints_all[:],
                in_=ints_p[:, :, :].rearrange("b p c -> p b c"))
            for b in range(nblk):
                for i in range(2):
                    nc.gpsimd.indirect_dma_start(
                        out=keS_all[:, (2 * b + i) * P128:(2 * b + i + 1) * P128],
                        out_offset=None, in_=ke_d[:, :],
                        in_offset=bass.IndirectOffsetOnAxis(
                            ap=ints_all[:, b, 8 + i:9 + i], axis=0))

            for i in range(NTX4 // ABATCH):
                xt4 = sbA.tile([P128, ABATCH * P128], BF16, tag="xt4")
                nc.sync.dma_start(
                    out=xt4[:], in_=xT[:, i * ABATCH * P128:(i + 1) * ABATCH * P128])
                kv4 = sbA.tile([P128, ABATCH, 2 * D], BF16, tag="kv4")
                for k in range(ABATCH):
                    mm = psA.tile([P128, 2 * D], F32, space="PSUM", tag="psa")
                    nc.tensor.matmul(out=mm[:], lhsT=xt4[:, k * P128:(k + 1) * P128],
                                     rhs=w_cat[:], start=True, stop=True)
                    if k % 2 == 0:
                        nc.scalar.copy(out=kv4[:, k, :], in_=mm[:])
                    else:
                        nc.vector.tensor_copy(kv4[:, k, :], mm[:])
                dst = kv_d[i * ABATCH * P128:(i + 1) * ABATCH * P128, :]
                nc.sync.dma_start(
                    out=dst.rearrange("(k p) w -> p k w", p=P128), in_=kv4[:])

        tc.strict_bb_all_engine_barrier()

        # ---- phase B: per-block gather + attention + segment reduce ---------
        MOFF = 0                    # m8 offset in mm8
        TOFF = TPB * P128           # mt8 offset
        BOFF = 2 * TPB * P128       # bias offset
        G4 = 4                      # tiles per DVE batch group
        with tc.tile_pool(name="sbB", bufs=4) as sbB, \
             tc.tile_pool(name="sbB2", bufs=3) as sbB2, \
             tc.tile_pool(name="ps_ke", bufs=2, space="PSUM") as ps_ke, \
             tc.tile_pool(name="ps_q", bufs=2, space="PSUM") as ps_q, \
             tc.tile_pool(name="ps_acc", bufs=2, space="PSUM") as ps_acc, \
             tc.tile_pool(name="ps_epi", bufs=2, space="PSUM") as ps_epi:
            for b in range(nblk):
                mm8_sb = sbB2.tile([P128, 2 * TPB * P128 + 8], BF16, tag="mm8")
                nc.sync.dma_start(out=mm8_sb[:], in_=mm8_p[b, :, :])
                qblk = sbB2.tile([P128, D], BF16, tag="qblk")
                nc.sync.dma_start(out=qblk[:], in_=q_d[b * P128:(b + 1) * P128, :])

                acc = ps_acc.tile([P128, D + H], F32, space="PSUM", tag="acc")
                for g in range(TPB // G4):
                    kv_g4 = sbB.tile([P128, G4, 2 * D], BF16, tag="kv_g4")
                    for i in range(G4):
                        t = g * G4 + i
                        nc.gpsimd.indirect_dma_start(
                            out=kv_g4[:, i, :], out_offset=None, in_=kv_d[:, :],
                            in_offset=bass.IndirectOffsetOnAxis(ap=ints_all[:, b, t:t + 1], axis=0))
                    keX4 = ps_ke.tile([P128, G4, D], F32, space="PSUM", tag="keX4")
                    qx4 = ps_q.tile([P128, G4, D], F32, space="PSUM", tag="qx4")
                    keS = keS_all[:, (2 * b + g) * P128:(2 * b + g + 1) * P128]
                    for i in range(G4):
                        t = g * G4 + i
                        nc.tensor.matmul(
                            out=keX4[:, i, :],
                            lhsT=r4_sb[:, (t % 4) * P128:(t % 4 + 1) * P128],
                            rhs=keS, start=True, stop=True)
                        nc.tensor.matmul(
                            out=qx4[:, i, :],
                            lhsT=mm8_sb[:, TOFF + t * P128:TOFF + (t + 1) * P128],
                            rhs=qblk[:], start=True, stop=True)

                    kk4 = sbB.tile([P128, G4, D], F32, tag="kk4")
                    nc.vector.tensor_add(kk4[:], kv_g4[:, :, 0:D], keX4[:])
                    prod4 = sbB.tile([P128, G4, D], F32, tag="prod4")
                    nc.vector.tensor_mul(prod4[:], kk4[:], qx4[:])
                    sc4 = sbB.tile([P128, G4 * H], F32, tag="sc4")
                    nc.vector.tensor_reduce(
                        out=sc4[:], in_=prod4[:].rearrange("p g (h d) -> p (g h) d", h=H),
                        axis=mybir.AxisListType.X, op=mybir.AluOpType.add)
                    scb4 = sbB.tile([P128, G4, H], F32, tag="scb4")
                    nc.vector.tensor_tensor(
                        out=scb4[:], in0=sc4[:].rearrange("p (g h) -> p g h", g=G4),
                        in1=mm8_sb[:, BOFF + g * G4:BOFF + (g + 1) * G4]
                            .rearrange("p (g o) -> p g o", o=1).to_broadcast([P128, G4, H]),
                        op=mybir.AluOpType.add)
                    wex4 = sbB.tile([P128, G4, D + H], BF16, tag="wex4")
                    ex4 = wex4[:, :, D:D + H]
                    nc.scalar.activation(
                        out=ex4, in_=scb4[:], func=mybir.ActivationFunctionType.Exp,
                        bias=0.0, scale=1.0)
                    nc.vector.tensor_tensor(
                        out=wex4[:, :, 0:D].rearrange("p g (h d) -> p g h d", h=H),
                        in0=kv_g4[:, :, D:2 * D].rearrange("p g (h d) -> p g h d", h=H),
                        in1=ex4.rearrange("p g (h o) -> p g h o", o=1)
                            .to_broadcast([P128, G4, H, DH]),
                        op=mybir.AluOpType.mult)

                    for i in range(G4):
                        t = g * G4 + i
                        # single matmul per tile: PSUM `start` clears
                        # has_written bank-wide -> NUM+DEN in one group
                        nc.tensor.matmul(
                            out=acc[:],
                            lhsT=mm8_sb[:, MOFF + t * P128:MOFF + (t + 1) * P128],
                            rhs=wex4[:, i, :], start=(t == 0), stop=(t == TPB - 1))

                den = sbB2.tile([P128, H], F32, tag="den")
                nc.vector.tensor_scalar_add(out=den[:], in0=acc[:, D:D + H],
                                            scalar1=1e-30)
                denr = sbB2.tile([P128, H], F32, tag="denr")
                nc.vector.reciprocal(denr[:], den[:])
                ctx_sb = sbB2.tile([P128, D], F32, tag="ctx")
                nc.vector.tensor_tensor(
                    out=ctx_sb[:].rearrange("p (h d) -> p h d", h=H),
                    in0=acc[:, 0:D].rearrange("p (h d) -> p h d", h=H),
                    in1=denr[:].rearrange("p (h o) -> p h o", o=1).to_broadcast([P128, H, DH]),
                    op=mybir.AluOpType.mult)
                ctxT_ps = ps_epi.tile([P128, D], F32, space="PSUM", tag="epi")
                nc.tensor.transpose(out=ctxT_ps[:], in_=ctx_sb[:], identity=ident_f[:])
                ctxT_sb = sbB2.tile([P128, D], F32, tag="ctxT_sb")
                nc.scalar.copy(out=ctxT_sb[:], in_=ctxT_ps[:])
                o_ps = ps_epi.tile([P128, D], F32, space="PSUM", tag="epi")
                nc.tensor.matmul(out=o_ps[:], lhsT=ctxT_sb[:], rhs=wo_sb[:],
                                 start=True, stop=True)
                o_sb = sbB2.tile([P128, D], F32, tag="o_sb")
                nc.vector.tensor_add(o_sb[:], o_ps[:], bo2_sb[:])
                o_relu = sbB2.tile([P128, D], F32, tag="o_relu")
                nc.scalar.activation(out=o_relu[:], in_=o_sb[:],
                                     func=mybir.ActivationFunctionType.Relu)
                nc.sync.dma_start(out=out[b * P128:(b + 1) * P128, :], in_=o_relu[:])

    nc.compile()
    return nc


_CACHE = {}


def _get_nc(nblk):
    if nblk not in _CACHE:
        _CACHE[nblk] = _build_nc(nblk)
    return _CACHE[nblk]


def kernel(**inputs):
    import ml_dtypes
    x = np.ascontiguousarray(np.asarray(inputs["x"], np.float32))
    ea = np.ascontiguousarray(np.asarray(inputs["edge_attr"], np.float32))
    owners = np.asarray(inputs["owners"], np.int32)
    pair_e = np.asarray(inputs["pair_e"], np.int32)
    pair_u = np.asarray(inputs["pair_u"], np.int32)

    nblk, per_core, node_map = _route(owners, pair_e, pair_u)
    nc = _get_nc(nblk)
    ntq = -(-nblk // ABATCH) * ABATCH
    padq = ntq * P128

    bf = ml_dtypes.bfloat16
    xT = np.zeros((P128, PADN), bf)
    xT[:, :N] = x.T.astype(bf)
    eaT = np.zeros((ED, PADE), bf)
    eaT[:, :E] = ea.T.astype(bf)

    def trep(b):
        return np.tile(np.asarray(b, np.float32)[None, :], (P128, 1))

    shared = dict(
        xT=xT, eaT=eaT,
        w_linT=np.ascontiguousarray(np.asarray(inputs["w_lin"], np.float32).T),
        w_eT=np.ascontiguousarray(np.asarray(inputs["w_e"], np.float32).T),
        w_q=np.asarray(inputs["w_q"], np.float32),
        w_k=np.asarray(inputs["w_k"], np.float32),
        w_v=np.asarray(inputs["w_v"], np.float32),
        w_o=np.asarray(inputs["w_o"], np.float32),
        bv_c=np.asarray(inputs["b_v"], np.float32).reshape(D, 1),
        bq_m=trep(inputs["b_q"]), bo_m=trep(inputs["b_o"]),
    )
    # R4[s, t*128 + j] = 1 iff s == 32*t + j//4  (slot -> pair expansion)
    r4 = np.zeros((P128, 4, P128), np.float32)
    jj = np.arange(P128)
    for tm in range(4):
        r4[32 * tm + jj // 4, tm, jj] = 1.0
    r4 = r4.reshape(P128, 4 * P128).astype(bf)
    in_maps = []
    for c in range(NC):
        ints, mm8, qcols = per_core[c]
        xqT = np.zeros((P128, padq), bf)
        valid = qcols >= 0
        xqT[:, np.nonzero(valid)[0]] = x[qcols[valid]].T.astype(bf)
        in_maps.append(dict(shared, xqT=xqT, ints_p=ints, mm8_p=mm8, r4_p=r4))

    import os
    trace = os.environ.get("KERNEL_TRACE", "0") == "1"
    kwargs = {}
    if trace:
        kwargs = dict(trace=True, tmpdir=os.environ.get("KERNEL_TRACE_DIR") or None)
    res = run_bass_kernel_spmd(nc, in_maps, core_ids=list(range(NC)), **kwargs)
    global _LAST_RESULTS
    _LAST_RESULTS = res

    out_full = np.zeros((N, D), np.float32)
    for c in range(NC):
        oc = res.results[c]["out"]
        valid = node_map[c] >= 0
        out_full[node_map[c][valid]] = oc[valid]
    return out_full
